# revision 2
# baseline (speedup 1.0000x reference)
"""BitLinear (RMSNorm + per-token int8 act quant + ternary weight quant + matmul)
on 8 Trainium2 NeuronCores, token-parallel.

Math notes:
  activation_quant: q = round(xn * s_t), s_t = 127/(amax(|xn|)+eps); xq = q/s_t.
  weight_quant:     w3 = clip(round(w*s_w), -1, 1), s_w = 1/(mean|w|+eps); wq = w3/s_w.
  out = xq @ wq.T = (q @ w3.T) * (1/s_t) * (mean|w|+eps)
  q in [-127,127] and w3 in {-1,0,1} are exactly representable in bf16, and the
  f32 PSUM accumulation of their products (<2^24) is exact, so the big matmul
  runs at full bf16 TensorE rate with zero quantization-side error.
  round() is implemented exactly (RNE, matches jnp.round) via the fp32
  magic-constant trick: (v + 1.5*2^23) - 1.5*2^23.

Per-core program (Tc=2048 tokens, D=2048, F=8192):
  stage A: stream wA (this core's F/8 slice), abs-sum reduce -> mean|w| via
           AllReduce + ones-matmul partition reduce
  stage X: stream x [t,d], per-token sum(x^2) and amax(|x|) column stats
  stage B: per-token multiplier beta = 127/(rms*(amax+eps)) transposed to a
           row via PE transpose and broadcast to [128, Tc] via k=1 ones-matmul
  stage Q: stream x.T [d,t], qT = round(xT * beta) -> bf16, SBUF-resident
  stage M: for each 1024-wide feature group: ternarize w.T tiles (ACT round +
           DVE clip) -> bf16 (double-buffered); then per token tile one
           [128,1024] 2-bank PSUM accumulation with the stationary qT[j,ti]
           held across the two 512-col matmuls (redundant LDWEIGHTS stripped
           post-schedule), scaled ACT drain, DMA out.
"""
import numpy as np
import concourse.bacc as bacc
import concourse.tile as tile
import concourse.mybir as mybir
from concourse import masks
from concourse.bass_utils import run_bass_kernel_spmd

Alu = mybir.AluOpType
Act = mybir.ActivationFunctionType
F32 = mybir.dt.float32
BF16 = mybir.dt.bfloat16

C = 1.5 * 2.0 ** 23  # fp32 RNE rounding constant
EPS_NORM = 1e-6
EPS_ACT = 1e-5
EPS_W = 1e-5
P = 128          # partitions
FCH = 512        # matmul free-dim chunk (one PSUM bank of f32)
FG = 1024        # stage-M feature group width (2 PSUM banks)

NCORES = 8


def _strip_redundant_ldweights(nc):
    """Remove InstLdweights that reload the exact AP the PE already holds
    (consecutive same-stationary matmuls) and carry no sync. Runs after tile
    scheduling/legalization, before bacc.compile(), so wait migration in
    move_matmul_waits_to_ldweights lands on the surviving ldweights."""
    n = 0
    for blk in nc.m.functions[0].blocks:
        keep = []
        prev_key = None
        for inst in blk.instructions:
            nm = type(inst).__name__
            if nm == "InstMatmult":
                keep.append(inst)
                continue
            if nm == "InstLdweights":
                key = str(inst.ins[0])
                si = inst.sync_info
                clean = si is None or (not si.on_wait and not si.on_update)
                if key == prev_key and clean:
                    n += 1
                    continue
                prev_key = key
                keep.append(inst)
                continue
            prev_key = None
            keep.append(inst)
        blk.instructions[:] = keep
    return n


def build_program(Tc, D, F, n_devices=NCORES, g_is_ones=True):
    nT = Tc // P     # token tiles
    nD = D // P      # contraction blocks
    FA = F // n_devices if n_devices > 1 else F
    nG = F // FG     # stage-M feature groups
    nCh = FG // FCH

    def chunks(total, sz):
        return [(st, min(sz, total - st)) for st in range(0, total, sz)]

    nc = bacc.Bacc("TRN2", num_devices=n_devices)
    x = nc.dram_tensor("x", [Tc, D], F32, kind="ExternalInput")
    xT = nc.dram_tensor("xT", [D, Tc], F32, kind="ExternalInput")
    wT = nc.dram_tensor("wT", [D, F], F32, kind="ExternalInput")
    g = nc.dram_tensor("g", [1, D], F32, kind="ExternalInput")
    wA = nc.dram_tensor("wA", [D, FA], F32, kind="ExternalInput")
    out = nc.dram_tensor("out", [Tc, F], F32, kind="ExternalOutput")
    done = nc.dram_tensor("done", [1, 8], F32, kind="ExternalOutput")
    cc_in = nc.dram_tensor("cc_in", [P, 1], F32)
    cc_out = nc.dram_tensor("cc_out", [P, 1], F32)

    nH = 2 if nT % 2 == 0 else 1
    hT = nT // nH          # token tiles per half
    hW = hT * P            # token columns per half

    with tile.TileContext(nc) as tc:
        with tc.tile_pool(name="const", bufs=1) as const_pool, \
             tc.tile_pool(name="stats", bufs=1) as stats_pool, \
             tc.tile_pool(name="qres", bufs=1) as qres_pool, \
             tc.tile_pool(name="wa", bufs=2) as wa_pool, \
             tc.tile_pool(name="xi", bufs=2) as xi_pool, \
             tc.tile_pool(name="xtj", bufs=2) as xtj_pool, \
             tc.tile_pool(name="wb", bufs=3) as wb_pool, \
             tc.tile_pool(name="wr", bufs=2) as wr_pool, \
             tc.tile_pool(name="w3", bufs=2) as w3_pool, \
             tc.tile_pool(name="osb", bufs=2) as osb_pool, \
             tc.tile_pool(name="ps_small", bufs=1, space="PSUM") as pss, \
             tc.tile_pool(name="ps_out", bufs=2, space="PSUM") as pso:

            # ---- constants ----
            identf = const_pool.tile([P, P], F32)
            masks.make_identity(nc, identf[:])
            ones_col = const_pool.tile([P, P], F32)
            nc.vector.memset(ones_col[:], 1.0)
            ones_row = const_pool.tile([1, P], F32)
            nc.vector.memset(ones_row[:], 1.0)
            if not g_is_ones:
                gT = const_pool.tile([P, nD], F32)
                nc.sync.dma_start(gT[:], g.ap().rearrange("a (c p) -> (a p) c", p=P))
                g_bc = const_pool.tile([P, D], F32)
                g_row = const_pool.tile([1, D], F32)
                nc.sync.dma_start(g_row[:], g.ap())
                for st, w in chunks(D, FCH):
                    pgb = pss.tile([P, FCH], F32, tag="pgb")
                    nc.tensor.matmul(pgb[:, :w], ones_row[:], g_row[:, st:st + w],
                                     start=True, stop=True)
                    nc.scalar.activation(g_bc[:, st:st + w], pgb[:, :w],
                                         Act.Copy, bias=0.0, scale=1.0)

            # persistent per-token stats (column layout [P, nT])
            wmeane = stats_pool.tile([P, 1], F32)
            swinv = stats_pool.tile([P, 1], F32)
            rowscale = stats_pool.tile([P, nT], F32)
            s1 = stats_pool.tile([P, nT], F32)    # ssum -> rinv -> beta
            s2 = stats_pool.tile([P, nT], F32)    # amax -> ae -> 1/ae
            beta_row = stats_pool.tile([1, Tc], F32)
            beta_bc = stats_pool.tile([P, Tc], F32)
            qT = []
            for j in range(nD):
                qTj = qres_pool.tile([P, Tc], BF16, tag=f"qT{j}")
                qT.append(qTj)

            # ---- stage A: mean|w| over this core's F/8 slice of wA via ACT
            # Abs+accum_out, AllReduce across cores, ones-matmul bcast ----
            acc_a = stats_pool.tile([P, nD], F32)
            for j in range(nD):
                wa = wa_pool.tile([P, FA], F32)
                nc.sync.dma_start(wa[:], wA.ap()[j * P:(j + 1) * P, :])
                wascr = wr_pool.tile([P, FG], F32, tag="wr")
                nc.scalar.activation(wascr[:, :FA], wa[:], Act.Abs, bias=0.0,
                                     scale=1.0, accum_out=acc_a[:, j:j + 1])
            acc1 = stats_pool.tile([P, 1], F32)
            nc.vector.tensor_reduce(acc1[:], acc_a[:], axis=mybir.AxisListType.X,
                                    op=Alu.add)
            if n_devices > 1:
                nc.sync.dma_start(cc_in.ap(), acc1[:])
                nc.gpsimd.collective_compute(
                    "AllReduce", Alu.add,
                    replica_groups=[list(range(n_devices))],
                    ins=[cc_in.ap().opt()],
                    outs=[cc_out.ap().opt()],
                )
                ccred = stats_pool.tile([P, 1], F32)
                nc.sync.dma_start(ccred[:], cc_out.ap())
            else:
                ccred = acc1
            ptot = pss.tile([P, 1], F32, tag="ptot")
            nc.tensor.matmul(ptot[:], ones_col[:], ccred[:], start=True, stop=True)
            nc.scalar.activation(wmeane[:], ptot[:], Act.Copy,
                                 bias=float(EPS_W), scale=1.0 / float(D * F))
            nc.vector.reciprocal(swinv[:], wmeane[:])   # = s_w

            # ---- stages X/Beta/Q, pipelined in token halves ----
            for h in range(nH):
                for i in range(h * hT, (h + 1) * hT):
                    xi = xi_pool.tile([P, D], F32, tag="xi")
                    nc.sync.dma_start(xi[:], x.ap()[i * P:(i + 1) * P, :])
                    if g_is_ones:
                        nc.vector.tensor_reduce(s2[:, i:i + 1], xi[:],
                                                axis=mybir.AxisListType.X,
                                                op=Alu.max,
                                                apply_absolute_value=True)
                        # in-place x^2 scratch on ScalarE, accumulating sum(x^2)
                        nc.scalar.activation(xi[:], xi[:], Act.Square, bias=0.0,
                                             scale=1.0, accum_out=s1[:, i:i + 1])
                    else:
                        xg = wr_pool.tile([P, max(FG, D)], F32, tag="wrg")
                        nc.vector.tensor_tensor(xg[:, :D], xi[:], g_bc[:],
                                                op=Alu.mult)
                        nc.vector.tensor_reduce(s2[:, i:i + 1], xg[:, :D],
                                                axis=mybir.AxisListType.X,
                                                op=Alu.max,
                                                apply_absolute_value=True)
                        nc.scalar.activation(xi[:], xi[:], Act.Square, bias=0.0,
                                             scale=1.0, accum_out=s1[:, i:i + 1])

                # per-token scalar chain on this half, in place
                a, b = h * hT, (h + 1) * hT
                s1h, s2h = s1[:, a:b], s2[:, a:b]
                nc.vector.tensor_scalar(s1h, s1h, 1.0 / float(D), float(EPS_NORM),
                                        op0=Alu.mult, op1=Alu.add)
                nc.scalar.activation(s1h, s1h, Act.Sqrt, bias=0.0, scale=1.0)
                nc.vector.reciprocal(s1h, s1h)          # rinv
                nc.vector.tensor_tensor(s2h, s2h, s1h, op=Alu.mult)
                nc.vector.tensor_scalar(s2h, s2h, float(EPS_ACT), None,
                                        op0=Alu.add)    # ae = amax_n + eps
                nc.vector.tensor_scalar(rowscale[:, a:b], s2h, wmeane[:, 0:1],
                                        1.0 / 127.0, op0=Alu.mult, op1=Alu.mult)
                nc.vector.reciprocal(s2h, s2h)
                nc.vector.tensor_tensor(s1h, s1h, s2h, op=Alu.mult)
                nc.vector.tensor_scalar(s1h, s1h, 127.0, None, op0=Alu.mult)

                # beta column -> row (PE transpose), broadcast to [P, half]
                for i in range(a, b):
                    prow = pss.tile([1, P], F32, tag="prow")
                    nc.tensor.transpose(prow[:], s1[:, i:i + 1], identf[:])
                    nc.scalar.activation(beta_row[:, i * P:(i + 1) * P], prow[:],
                                         Act.Copy, bias=0.0, scale=1.0)
                for st, w in chunks(hW, FCH):
                    st = h * hW + st
                    pbb = pss.tile([P, FCH], F32, tag="pgb")
                    nc.tensor.matmul(pbb[:, :w], ones_row[:], beta_row[:, st:st + w],
                                     start=True, stop=True)
                    nc.scalar.activation(beta_bc[:, st:st + w], pbb[:, :w],
                                         Act.Copy, bias=0.0, scale=1.0)

                # qT for this token half (bf16, SBUF resident)
                for j in range(nD):
                    xtj = xtj_pool.tile([P, hW], F32, tag="xtj")
                    nc.sync.dma_start(xtj[:], xT.ap()[j * P:(j + 1) * P,
                                                      h * hW:(h + 1) * hW])
                    if g_is_ones:
                        nc.vector.tensor_tensor(xtj[:], xtj[:],
                                                beta_bc[:, h * hW:(h + 1) * hW],
                                                op=Alu.mult)
                    else:
                        nc.scalar.activation(xtj[:], xtj[:], Act.Copy,
                                             bias=0.0, scale=gT[:, j:j + 1])
                        nc.vector.tensor_tensor(xtj[:], xtj[:],
                                                beta_bc[:, h * hW:(h + 1) * hW],
                                                op=Alu.mult)
                    nc.vector.tensor_scalar(qT[j][:, h * hW:(h + 1) * hW],
                                            xtj[:], C, C,
                                            op0=Alu.add, op1=Alu.subtract)

            # ---- stage M: ternarize + matmul (stationary held over the
            # feature group) + scaled drain + store ----
            for fg in range(nG):
                w3 = []
                for j in range(nD):
                    wb = wb_pool.tile([P, FG], F32)
                    nc.sync.dma_start(wb[:], wT.ap()[j * P:(j + 1) * P,
                                                     fg * FG:(fg + 1) * FG])
                    wr = wr_pool.tile([P, FG], F32, tag="wr")
                    # wr = C + round(w * s_w)
                    nc.scalar.activation(wr[:], wb[:], Act.Copy,
                                         bias=C, scale=swinv[:, 0:1])
                    w3j = w3_pool.tile([P, FG], BF16, tag=f"w3_{j}")
                    # (wr - C) clipped to [-1, 1], cast bf16
                    nc.vector.tensor_scalar(wr[:], wr[:], C, -1.0,
                                            op0=Alu.subtract, op1=Alu.max)
                    nc.vector.tensor_scalar(w3j[:], wr[:], 1.0, None, op0=Alu.min)
                    w3.append(w3j)
                for ti in range(nT):
                    pout = pso.tile([P, FG], F32)
                    for j in range(nD):
                        for fc in range(nCh):
                            nc.tensor.matmul(
                                pout[:, fc * FCH:(fc + 1) * FCH],
                                qT[j][:, ti * P:(ti + 1) * P],
                                w3[j][:, fc * FCH:(fc + 1) * FCH],
                                start=(j == 0), stop=(j == nD - 1))
                    ost = osb_pool.tile([P, FG], F32)
                    nc.scalar.activation(ost[:], pout[:], Act.Copy, bias=0.0,
                                         scale=rowscale[:, ti:ti + 1])
                    nc.sync.dma_start(out.ap()[ti * P:(ti + 1) * P,
                                               fg * FG:(fg + 1) * FG], ost[:])

            dsb = stats_pool.tile([1, 8], F32)
            nc.vector.memset(dsb[:], 1.0)
            nc.sync.dma_start(done.ap(), dsb[:])

    _strip_redundant_ldweights(nc)
    nc.compile()
    return nc


_prog_cache = {}


def _get_program(Tc, D, F, g_is_ones=True):
    key = (Tc, D, F, g_is_ones)
    if key not in _prog_cache:
        _prog_cache[key] = build_program(Tc, D, F, g_is_ones=g_is_ones)
    return _prog_cache[key]


def make_in_maps(x, norm_weight, weight):
    B, S, D = x.shape
    F = weight.shape[0]
    T = B * S
    Tc = T // NCORES
    xf = np.ascontiguousarray(x.reshape(T, D), dtype=np.float32)
    wTv = np.ascontiguousarray(weight.T).astype(np.float32, copy=False)
    gv = np.ascontiguousarray(norm_weight.reshape(1, D), dtype=np.float32)
    FA = F // NCORES
    in_maps = []
    for c in range(NCORES):
        xs = xf[c * Tc:(c + 1) * Tc]
        in_maps.append({
            "x": xs,
            "xT": np.ascontiguousarray(xs.T),
            "wT": wTv,
            "wA": np.ascontiguousarray(wTv[:, c * FA:(c + 1) * FA]),
            "g": gv,
        })
    return in_maps, (B, S, T, Tc, D, F)


def kernel(x, norm_weight, weight):
    x = np.asarray(x)
    norm_weight = np.asarray(norm_weight)
    weight = np.asarray(weight)
    in_maps, (B, S, T, Tc, D, F) = make_in_maps(x, norm_weight, weight)
    nc = _get_program(Tc, D, F, g_is_ones=bool(np.all(norm_weight == 1.0)))
    res = None
    last_err = None
    for _ in range(3):
        # the walrus backend has a rare nondeterministic ISA-check race;
        # a clean retry recompiles and passes
        try:
            res = run_bass_kernel_spmd(nc, in_maps, core_ids=list(range(NCORES)))
            break
        except Exception as e:
            last_err = e
    if res is None:
        raise last_err
    outp = np.concatenate([res.results[c]["out"] for c in range(NCORES)], axis=0)
    return np.ascontiguousarray(outp.reshape(B, S, F).astype(np.float32, copy=False))
